# revision 2
# baseline (speedup 1.0000x reference)
"""Trainium2 kernel for the 101-layer scalar-affine+ReLU chain.

The reference applies h -> relu(w_i * h + b_i) for i = 0..100 elementwise on a
(32, 1, 1024, 1024) f32 tensor. Each step is x -> max(0, w*x + b); for w >= 0
the composition of such maps stays in the closed form

    F(x) = max(C, A*x + D)

with the recursion  C' = max(0, w*C + b),  A' = w*A,  D' = w*D + b  (start
C = -inf, A = 1, D = 0).  So the whole chain is one clamp-affine, and the
kernel is a single memory-bound elementwise pass:

    out = relu(A*x + (D - C)) + C

The pass is HBM-bound (358 GB/s per core), so the I/O is done in fp16: the
host quantizes x to fp16 (error ~2^-11, vs the 2e-2 rel-err budget), the
device streams fp16 in / fp16 out (half the HBM traffic of f32), and the
host widens the result back to f32.

Sharding: pure data parallel, batch 32 split 4-per-core across 8 cores.
Per core: load 8 MiB fp16, one ACT pass + one DVE pass in SBUF, store 8 MiB.
"""

import numpy as np

N_CORES = 8
FULL_SHAPE = (32, 1, 1024, 1024)
PER_CORE_ELEMS = (FULL_SHAPE[0] // N_CORES) * FULL_SHAPE[1] * FULL_SHAPE[2] * FULL_SHAPE[3]

P = 128          # SBUF partitions
FREE = 4096      # free-dim elements per tile  (fp16 tile = 128*4096*2B = 1 MiB)
NT = PER_CORE_ELEMS // (P * FREE)  # tiles per core

_nc_cache = {}


def _collapse(w, b):
    """Fold the relu-affine chain into (A, D, C) with F(x) = max(C, A*x + D)."""
    a = np.float64(1.0)
    d = np.float64(0.0)
    c = -np.inf
    for wi, bi in zip(w.astype(np.float64), b.astype(np.float64)):
        c = max(0.0, float(wi * c + bi))
        a = wi * a
        d = wi * d + bi
    return float(a), float(d), float(c)


def _build(A, D, C, iters=None, free=FREE, bufs=4, dma_only=False):
    """Build the bass program. iters=None -> single pass (the real kernel);
    iters=k -> the same pass wrapped in a device-side For_i loop, used only
    by the timing harness (slope over k cancels host/RPC overhead)."""
    import concourse.bacc as bacc
    import concourse.mybir as mybir
    from concourse.tile import TileContext

    nt = PER_CORE_ELEMS // (P * free)
    # Bacc (not raw Bass): its finalize() runs generate_event_semaphores,
    # which splits multi-sem waits to satisfy TRN2's 1-wait-per-instruction
    # hardware constraint.
    nc = bacc.Bacc("TRN2", target_bir_lowering=False)
    x = nc.dram_tensor("x", [nt * P, free], mybir.dt.float16, kind="ExternalInput")
    y = nc.dram_tensor("y", [nt * P, free], mybir.dt.float16, kind="ExternalOutput")
    relu = mybir.ActivationFunctionType.Relu

    # Materialize the ACT bias constant outside the Tile program, behind a
    # barrier (same pattern Bass.__init__ uses for its 0.0/1.0 const APs), so
    # the Activation instructions don't pick up an extra sync wait.
    bias_tensor = nc.alloc_sbuf_tensor("bias_dc", [P, 1], mybir.dt.float32)
    nc.gpsimd.memset(bias_tensor.ap(), float(D - C))
    nc.all_engine_barrier()
    bias_t = bias_tensor.ap()

    with TileContext(nc) as tc:
        with (
            tc.tile_pool(name="ld", bufs=bufs) as ld_pool,
            tc.tile_pool(name="st", bufs=bufs) as st_pool,
        ):
            def one_pass():
                for i in range(nt):
                    t = ld_pool.tile([P, free], mybir.dt.float16)
                    nc.sync.dma_start(t[:], x[i * P:(i + 1) * P, :])
                    if dma_only:
                        nc.sync.dma_start(y[i * P:(i + 1) * P, :], t[:])
                        continue
                    o = st_pool.tile([P, free], mybir.dt.float16)
                    # o = relu(A*x + (D - C))
                    nc.scalar.activation(o[:], t[:], relu, bias=bias_t[:, :1], scale=float(A))
                    # o += C  ->  o = max(C, A*x + D)
                    nc.vector.tensor_scalar_add(o[:], o[:], float(C))
                    nc.sync.dma_start(y[i * P:(i + 1) * P, :], o[:])

            if iters is None:
                one_pass()
            else:
                with tc.For_i(0, iters, 1):
                    one_pass()
    nc.finalize()
    return nc


def _make_shards(x_f32):
    """Full f32 (32,1,1024,1024) -> 8 contiguous fp16 shards [NT*P, FREE]."""
    xh = np.ascontiguousarray(x_f32, dtype=np.float32).astype(np.float16)
    shards = xh.reshape(N_CORES, NT * P, FREE)
    return [np.ascontiguousarray(shards[k]) for k in range(N_CORES)]


def _run_device(x, A, D, C, trace=False):
    from concourse.bass_utils import run_bass_kernel_spmd

    key = (round(A, 12), round(D, 12), round(C, 12))
    nc = _nc_cache.get(key)
    if nc is None:
        nc = _build(A, D, C)
        _nc_cache[key] = nc

    in_maps = [{"x": s} for s in _make_shards(x)]
    try:
        res = run_bass_kernel_spmd(nc, in_maps, list(range(N_CORES)), trace=trace)
    except Exception:
        # The axon-tunneled devices occasionally come up wedged from a prior
        # interrupted session (NRT_EXEC_UNIT_UNRECOVERABLE); one retry after a
        # short pause reliably recovers.
        import time
        time.sleep(15)
        res = run_bass_kernel_spmd(nc, in_maps, list(range(N_CORES)), trace=trace)
    out = np.concatenate(
        [res.results[k]["y"].astype(np.float32).reshape(
            FULL_SHAPE[0] // N_CORES, *FULL_SHAPE[1:])
         for k in range(N_CORES)],
        axis=0,
    )
    return out, res


def kernel(x, w, b, trace=False, _return_res=False):
    x = np.ascontiguousarray(np.asarray(x, dtype=np.float32))
    w = np.asarray(w, dtype=np.float32)
    b = np.asarray(b, dtype=np.float32)
    assert x.shape == FULL_SHAPE, x.shape

    if np.any(w < 0.0):
        # Not reachable for the given distribution (w ~ N(1, 0.02^2)); exact
        # host fallback to keep the kernel correct for arbitrary params.
        h = x.copy()
        for wi, bi in zip(w, b):
            h = np.maximum(h * wi + bi, np.float32(0.0)).astype(np.float32)
        return h

    A, D, C = _collapse(w, b)
    out, res = _run_device(x, A, D, C, trace=trace)
    out = out.astype(np.float32, copy=False)
    if _return_res:
        return out, res
    return out


# revision 21
# speedup vs baseline: 2.7483x; 2.7483x over previous
"""Trainium2 kernel for the 101-layer scalar-affine+ReLU chain.

The reference applies h -> relu(w_i * h + b_i) for i = 0..100 elementwise on a
(32, 1, 1024, 1024) f32 tensor. Each step is x -> max(0, w*x + b); for w >= 0
the composition of such maps stays in the closed form

    F(x) = max(C, A*x + D)

with the recursion  C' = max(0, w*C + b),  A' = w*A,  D' = w*D + b  (start
C = -inf, A = 1, D = 0).  So the whole chain is one clamp-affine, and the
kernel is a single memory-bound elementwise pass:

    out = relu(A*x + (D - C)) + C

The pass is HBM-bound (358 GB/s per core), so I/O precision is traded for
bandwidth within the 2e-2 rel-err budget: the host quantizes x to fp16
(error ~2^-11), and the device computes the clamp-affine and writes the
result log-quantized to uint8 (q = round(ln(out/C)/s), a 256-level
geometric grid -> half-step rel err ~0.9%), which the host decodes with a
256-entry LUT. Per-core HBM traffic drops from 32 MiB (f32) to 12 MiB.

Device pipeline per tile: DVE max (clamp) -> ACT Ln with its free affine
(one pass) -> DVE quantize to u8. Loads issue on the SP HWDGE ring, stores
on the ACT ring so they don't queue behind pending loads.

Sharding: pure data parallel, batch 32 split 4-per-core across 8 cores.
_plan() simulates the pipeline's error on a host subsample and falls back
to fp16 or f32 I/O if the (w, b) at hand ever made u8 too coarse.
"""

import numpy as np

N_CORES = 8
FULL_SHAPE = (32, 1, 1024, 1024)
PER_CORE_ELEMS = (FULL_SHAPE[0] // N_CORES) * FULL_SHAPE[1] * FULL_SHAPE[2] * FULL_SHAPE[3]

P = 128          # SBUF partitions
FREE = 4096      # free-dim elements per tile  (fp16 tile = 128*4096*2B = 1 MiB)
NT = PER_CORE_ELEMS // (P * FREE)  # tiles per core

_nc_cache = {}


def _collapse(w, b):
    """Fold the relu-affine chain into (A, D, C) with F(x) = max(C, A*x + D)."""
    a = np.float64(1.0)
    d = np.float64(0.0)
    c = -np.inf
    for wi, bi in zip(w.astype(np.float64), b.astype(np.float64)):
        c = max(0.0, float(wi * c + bi))
        a = wi * a
        d = wi * d + bi
    return float(a), float(d), float(c)


def _build(A, D, C, iters=None, free=FREE, bufs=4, dma_only=False,
           mode="f16", s=None, quant_round=True, probe=None, tail_split=None,
           chunk=None):
    """Build the bass program. iters=None -> single pass (the real kernel);
    iters=k -> the same pass wrapped in a device-side For_i loop, used only
    by the timing harness (slope over k cancels host/RPC overhead).

    mode="f16": fp16 in -> max(C, A*x+D) -> fp16 out.
    mode="u8":  fp16 in -> q = round(ln(max(C, A*x+D)/C)/s) -> uint8 out
                (log-quantized output, decoded on host via a 256-entry LUT;
                halves the store-side HBM traffic)."""
    import concourse.bacc as bacc
    import concourse.mybir as mybir
    from concourse.tile import TileContext

    nt = PER_CORE_ELEMS // (P * free)
    # Bacc (not raw Bass): its finalize() runs generate_event_semaphores,
    # which splits multi-sem waits to satisfy TRN2's 1-wait-per-instruction
    # hardware constraint.
    nc = bacc.Bacc("TRN2", target_bir_lowering=False)
    in_dt = mybir.dt.float32 if mode == "f32" else mybir.dt.float16
    out_dt = {"u8": mybir.dt.uint8, "f32": mybir.dt.float32}.get(mode, mybir.dt.float16)
    x = nc.dram_tensor("x", [nt * P, free], in_dt, kind="ExternalInput")
    y = nc.dram_tensor("y", [nt * P, free], out_dt, kind="ExternalOutput")
    relu = mybir.ActivationFunctionType.Relu
    ln_f = mybir.ActivationFunctionType.Ln

    # Materialize the ACT bias constant outside the Tile program, behind a
    # barrier (same pattern Bass.__init__ uses for its 0.0/1.0 const APs), so
    # the Activation instructions don't pick up an extra sync wait.
    bias_tensor = nc.alloc_sbuf_tensor("bias_dc", [P, 1], mybir.dt.float32)
    bias_val = float(D / C) if mode == "u8" else float(D - C)
    nc.gpsimd.memset(bias_tensor.ap(), bias_val)
    # Tiny pre-loop Ln so the ACT table set loads once outside the For_i body.
    warm = nc.alloc_sbuf_tensor("warm", [P, 1], mybir.dt.float32)
    if mode == "u8":
        nc.gpsimd.memset(warm.ap(), 1.0)
    nc.all_engine_barrier()
    if mode == "u8":
        nc.scalar.activation(warm.ap(), warm.ap(), ln_f, bias=bias_tensor.ap()[:, :1],
                             scale=float(A / C))
        nc.all_engine_barrier()
    bias_t = bias_tensor.ap()

    x0 = (C - D) / A  # clamp threshold: max(C, A*x+D) == A*max(x, x0) + D

    scratch_q = None
    if probe in ("dmaonly3", "dma3sc", "noquant"):
        scratch_q = nc.alloc_sbuf_tensor("scratch_q", [P, free], mybir.dt.uint8)
        nc.gpsimd.memset(scratch_q.ap(), 0)
        nc.all_engine_barrier()

    if isinstance(bufs, int):
        bufs = (bufs, bufs, bufs)

    with TileContext(nc) as tc:
        with (
            tc.tile_pool(name="ld", bufs=bufs[0]) as ld_pool,
            tc.tile_pool(name="mid", bufs=bufs[1]) as mid_pool,
            tc.tile_pool(name="st", bufs=bufs[2]) as st_pool,
        ):
            # u8 stores issue on the ACT HWDGE ring so they don't queue
            # behind pending loads in the SP ring (measured ~5us/pass win).
            st_eng = (nc.scalar if (mode == "u8" and probe is None)
                      or probe in ("stsc", "dma3sc") else nc.sync)

            def one_pass():
                for i in range(nt):
                    if (mode == "u8" and tail_split and i == nt - 1
                            and probe in (None, "stsc")):
                        c0 = 0
                        for wdt in tail_split:
                            u8_chain(i, c0, wdt)
                            c0 += wdt
                        assert c0 == free, (c0, free)
                        continue
                    if (mode == "u8" and chunk and probe in (None, "stsc")):
                        t = ld_pool.tile([P, free], in_dt)
                        nc.sync.dma_start(t[:], x[i * P:(i + 1) * P, :])
                        for c0 in range(0, free, chunk):
                            u8_chain(i, c0, chunk, t)
                        continue
                    t = ld_pool.tile([P, free], in_dt)
                    nc.sync.dma_start(t[:], x[i * P:(i + 1) * P, :])
                    if dma_only:
                        nc.sync.dma_start(y[i * P:(i + 1) * P, :], t[:])
                        continue
                    if mode in ("f16", "f32"):
                        o = st_pool.tile([P, free], in_dt)
                        # o = relu(A*x + (D - C))
                        nc.scalar.activation(o[:], t[:], relu, bias=bias_t[:, :1],
                                             scale=float(A))
                        # o += C  ->  o = max(C, A*x + D)
                        nc.vector.tensor_scalar_add(o[:], o[:], float(C))
                        nc.sync.dma_start(y[i * P:(i + 1) * P, :], o[:])
                    else:
                        if probe in ("dmaonly3", "dma3sc"):
                            st_eng.dma_start(y[i * P:(i + 1) * P, :], scratch_q.ap())
                            continue
                        if probe == "noact":
                            nc.vector.tensor_scalar_max(t[:], t[:], float(x0))
                            q = st_pool.tile([P, free], mybir.dt.uint8)
                            nc.vector.tensor_scalar(q[:], t[:], float(1.0 / s), 0.0,
                                                    mybir.AluOpType.mult,
                                                    mybir.AluOpType.max)
                            nc.sync.dma_start(y[i * P:(i + 1) * P, :], q[:])
                            continue
                        u8_chain(i, 0, free, t)

            def u8_chain(i, c0, w, t=None):
                """One load->max->ln->quantize->store chain on columns
                [c0, c0+w) of row-block i. t: already-loaded [P, free] block
                tile (compute on its [:, c0:c0+w] slice) or None (load)."""
                if t is None:
                    tc_tile = ld_pool.tile([P, w], in_dt)
                    ts = tc_tile[:]
                    nc.sync.dma_start(ts, x[i * P:(i + 1) * P, c0:c0 + w])
                else:
                    ts = t[:, c0:c0 + w]
                # m = max(x, x0) in place (clamped inputs -> z == 1)
                nc.vector.tensor_scalar_max(ts, ts, float(x0))
                u = mid_pool.tile([P, w], mybir.dt.float16)
                # u = ln((A/C)*m + D/C) = ln(out/C) in [0, 255*s]
                nc.scalar.activation(u[:], ts, ln_f, bias=bias_t[:, :1],
                                     scale=float(A / C))
                if probe == "noquant":
                    nc.sync.dma_start(y[i * P:(i + 1) * P, c0:c0 + w],
                                      scratch_q.ap()[:, c0:c0 + w])
                    return
                q = st_pool.tile([P, w], mybir.dt.uint8)
                qeng = nc.gpsimd if (probe == "gq" or
                                     (probe == "alt" and i % 2)) else nc.vector
                if quant_round:
                    # fp->u8 convert rounds to nearest (verified on HW)
                    qeng.tensor_scalar(q[:], u[:], float(1.0 / s), 0.0,
                                       mybir.AluOpType.mult,
                                       mybir.AluOpType.max)
                else:
                    qeng.tensor_scalar(q[:], u[:], float(1.0 / s), 0.5,
                                       mybir.AluOpType.mult,
                                       mybir.AluOpType.add)
                st_eng.dma_start(y[i * P:(i + 1) * P, c0:c0 + w], q[:])

            if iters is None:
                one_pass()
            else:
                with tc.For_i(0, iters, 1):
                    one_pass()
    nc.finalize()
    return nc


def _make_shards(x_f32, mode="u8"):
    """Full f32 (32,1,1024,1024) -> 8 contiguous per-core shards [NT*P, FREE]."""
    dt = np.float32 if mode == "f32" else np.float16
    xh = np.ascontiguousarray(x_f32, dtype=np.float32).astype(dt)
    shards = xh.reshape(N_CORES, NT * P, FREE)
    return [np.ascontiguousarray(shards[k]) for k in range(N_CORES)]


def _simulate_rel_err(xh, A, D, C, mode, s=None):
    """Max rel-err of the device pipeline (simulated on host) vs the exact
    collapsed map, on a subsample. xh: fp16 input subsample."""
    xs = xh.astype(np.float32)
    exact = np.maximum(C, np.float64(A) * xh.astype(np.float64) + np.float64(D))
    if mode == "f16":
        t = np.maximum(np.float32(A) * xs + np.float32(D - C),
                       np.float32(0)).astype(np.float16)
        out = (t.astype(np.float32) + np.float32(C)).astype(np.float16).astype(np.float64)
    elif mode == "u8":
        x0 = (C - D) / A
        m = np.maximum(xs, np.float32(x0))
        z = np.float32(A / C) * m + np.float32(D / C)
        u = np.log(np.maximum(z, np.float32(1e-37))).astype(np.float16)
        q = np.clip(np.rint(np.maximum(
            u.astype(np.float32) * np.float32(1.0 / s), np.float32(0.0))),
            0, 255).astype(np.uint8)
        out = np.float64(C) * np.exp(np.float64(s) * q.astype(np.float64))
    else:
        return 0.0
    denom = np.maximum(np.abs(exact), 1e-6)
    return float((np.abs(out - exact) / denom).max())


def _plan(x, w, b):
    """Choose the device pipeline (u8 / f16 / f32) and its parameters."""
    A, D, C = _collapse(w, b)
    xh = x.astype(np.float16)
    sample = xh.reshape(-1)[::5]
    out_max = float(A) * float(xh.max()) + float(D)
    if C > 1e-30 and out_max > C:
        s = float(np.log(out_max * 1.001 / C) / 255.0)
        if _simulate_rel_err(sample, A, D, C, "u8", s) < 1.5e-2:
            return A, D, C, "u8", s
    if _simulate_rel_err(sample, A, D, C, "f16") < 1.5e-2:
        return A, D, C, "f16", None
    return A, D, C, "f32", None


def _run_device(x, A, D, C, mode, s, trace=False):
    from concourse.bass_utils import run_bass_kernel_spmd

    key = (mode, round(A, 12), round(D, 12), round(C, 12),
           None if s is None else round(s, 14))
    nc = _nc_cache.get(key)
    if nc is None:
        nc = _build(A, D, C, mode=mode, s=s, bufs=6 if mode == "u8" else 4)
        _nc_cache[key] = nc

    in_maps = [{"x": sh} for sh in _make_shards(x, mode)]
    try:
        res = run_bass_kernel_spmd(nc, in_maps, list(range(N_CORES)), trace=trace)
    except Exception:
        # The axon-tunneled devices occasionally come up wedged from a prior
        # interrupted session (NRT_EXEC_UNIT_UNRECOVERABLE); one retry after a
        # short pause reliably recovers.
        import time
        time.sleep(15)
        res = run_bass_kernel_spmd(nc, in_maps, list(range(N_CORES)), trace=trace)
    ys = [res.results[k]["y"] for k in range(N_CORES)]
    if mode == "u8":
        lut = (np.float64(C) * np.exp(np.float64(s) * np.arange(256))).astype(np.float32)
        ys = [lut[yk] for yk in ys]
    out = np.concatenate(
        [yk.astype(np.float32).reshape(FULL_SHAPE[0] // N_CORES, *FULL_SHAPE[1:])
         for yk in ys],
        axis=0,
    )
    return out, res


def kernel(x, w, b, trace=False, _return_res=False):
    x = np.ascontiguousarray(np.asarray(x, dtype=np.float32))
    w = np.asarray(w, dtype=np.float32)
    b = np.asarray(b, dtype=np.float32)
    assert x.shape == FULL_SHAPE, x.shape

    if np.any(w < 0.0):
        # Not reachable for the given distribution (w ~ N(1, 0.02^2)); exact
        # host fallback to keep the kernel correct for arbitrary params.
        h = x.copy()
        for wi, bi in zip(w, b):
            h = np.maximum(h * wi + bi, np.float32(0.0)).astype(np.float32)
        return h

    A, D, C, mode, s = _plan(x, w, b)
    out, res = _run_device(x, A, D, C, mode, s, trace=trace)
    out = out.astype(np.float32, copy=False)
    if _return_res:
        return out, res
    return out
